# revision 8
# baseline (speedup 1.0000x reference)
"""ArcFace (AngularPenaltySMLoss) fused loss kernel for 8 Trainium2 NeuronCores.

Strategy: data-parallel over rows N (each core owns N/8 = 1024 rows of x,
streams the full W). Key design points:

  1. fp8(e4m3) DoubleRow matmul: W^T uploaded pre-transposed/pre-cast from the
     host as [128, 2, 10240] fp8 (scaled by SB, zero-padded 10000->10240), x
     normalized on-device and cast to fp8 x^T (scaled by SA, SA*SB = S = 30).
     Each matmul contracts the full K=256 (two 128-planes) in one instruction.
     The 240 zero pad classes contribute exactly exp(0)=1 on ACT-drained pairs
     (pad pairs are always 'A'); the tail subtracts the constant 240.
  2. j-outer main loop with a manually managed 4-slot PSUM ring
     (pm[128, 4, 1024] f32 = all 16KB).  Per row-block j, 10 class-chunks of
     1024 are drained in PAIRS by a single wide instruction over two adjacent
     slots [128, 2048]: 'A' pairs = ACT exp with fused row-sum accum_out;
     'V' pairs = DVE Schraudolph bit-trick exp (y=int32(A*v+B) is the f32 bit
     pattern of ~exp(v)) + DVE reduce of the bitcast.  4 slots + two drain
     engines keep the ring from stalling on heterogeneous drain rates.
  3. ACT runs ONLY Exp until the final Ln (one act-table switch at the tail).
     1/||x|| and sqrt(S^2-t^2) use the Quake rsqrt bit trick + 2 Newton steps
     on DVE; x*srinv casts and the target-dot run on GpSimd (Pool).
  4. Target path: host pre-gathers (W*SB)[target] as bf16 (data movement
     only); numerator = cosM*t_s - sinM*sqrt(S^2-t_s^2) on scaled t_s = S*t.
  5. Per-core partial sum of L_i; host combines 8 scalars.
"""

import math

import numpy as np

S = 30.0
MARGIN = 0.3
EPS = 1e-7
N, D, C = 8192, 256, 10000
NCORES = 8
NLOC = N // NCORES  # 1024 rows per core
NJ = NLOC // 128  # 8 row-chunks of 128 partitions
CP = 10240  # padded class count
CW = 1024  # class-chunk width (one PSUM slot)
NCH = CP // CW  # 10 chunks per row-block
NPAIR = NCH // 2  # 5 drain pairs per row-block
NPAD = CP - C  # 240 zero-pad classes -> exp contributes exactly NPAD
SA = 8.0  # fp8 scale folded into normalized x
SB = 3.75  # fp8 scale folded into W  (SA*SB = S)

# Schraudolph exp constants (f32 domain, int32 bit pattern), B tuned for
# zero exp-weighted mean error: B = 127*2^23 - round(0.0562*2^23)
AEXP = 12102203.0
BEXP = 1064881816.0
RSQRT_MAGIC = 1597463007.0  # 0x5f3759df

# If True, matmuls stream 1024 rhs rows (2x512) per instruction; fall back to
# 512 (2x256) if the toolchain rejects the wider moving AP.
WIDE_MM = True

# Drain-engine plan per (j, pair): 'A' = ACT, 'V' = DVE Schraudolph.
# Last pair of each j must be 'A' (owns the zero-pad classes). 29A:11V
# balances measured ACT (~2.15us/pair) vs DVE (~4.4us/pair + misc).
PAIR_PLAN = [
    "AVAAA",
    "AVAVA",
    "AVAAA",
    "AVAVA",
    "AVAAA",
    "AVAVA",
    "AVAAA",
    "AVAAA",
]

_CACHE = {}


def _build():
    import concourse.bass as bass  # noqa: F401
    import concourse.mybir as mybir
    import concourse.tile as tile
    from concourse import bacc
    from concourse.masks import make_identity

    f32 = mybir.dt.float32
    bf16 = mybir.dt.bfloat16
    f8 = mybir.dt.float8e4
    i32 = mybir.dt.int32
    AF = mybir.ActivationFunctionType
    OP = mybir.AluOpType
    DR = mybir.MatmulPerfMode.DoubleRow

    nc = bacc.Bacc()
    x_ext = nc.declare_dram_parameter("x", [128, NJ, D], f32, isOutput=False)
    wt_ext = nc.declare_dram_parameter("wt", [128, 2, CP], f8, isOutput=False)
    wg_ext = nc.declare_dram_parameter("wg", [128, NJ, D], bf16, isOutput=False)
    out_ext = nc.declare_dram_parameter("out", [1, 1], f32, isOutput=True)

    with tile.TileContext(nc) as tc:
        with (
            tc.tile_pool(name="singles", bufs=1) as singles,
            tc.tile_pool(name="idpool", bufs=2) as idpool,
            tc.tile_pool(name="pmain", bufs=1, space="PSUM") as psum_main,
        ):
            ident = singles.tile([128, 128], bf16)
            make_identity(nc, ident)

            # the whole PSUM: 4-slot ring + (pre-loop) bf16 transpose staging
            pm = psum_main.tile([128, 4, CW], f32)
            ptx = pm[:, 0, :].bitcast(bf16).rearrange("p (k q) -> p k q", q=128)

            # ---------------- loads ----------------
            xt = singles.tile([128, NJ, D], f32)
            nc.sync.dma_start(out=xt, in_=x_ext[:, :, :])
            wg = singles.tile([128, NJ, D], bf16)
            nc.sync.dma_start(out=wg, in_=wg_ext[:, :, :])
            wt = singles.tile([128, 2, CP], f8)
            for r in range(NPAIR):
                c0 = r * 2048
                nc.sync.dma_start(
                    out=wt[:, :, c0 : c0 + 2048], in_=wt_ext[:, :, c0 : c0 + 2048]
                )

            # ---------------- x normalization (DVE) ----------------
            ss = singles.tile([128, NJ], f32)
            sqd = singles.tile([128, D], bf16)
            for j in range(NJ):
                nc.vector.scalar_tensor_tensor(
                    out=sqd,
                    in0=xt[:, j, :],
                    scalar=1.0,
                    in1=xt[:, j, :],
                    op0=OP.mult,
                    op1=OP.mult,
                    accum_out=ss[:, j : j + 1],
                )

            def rsqrt2(src, fold=1.0, name=""):
                # Quake rsqrt + 2 Newton iterations; returns fold/sqrt(src)
                seed = singles.tile([128, NJ], i32, name=f"rs_seed{name}")
                nc.vector.tensor_scalar(
                    out=seed,
                    in0=src.bitcast(i32),
                    scalar1=-0.5,
                    scalar2=RSQRT_MAGIC,
                    op0=OP.mult,
                    op1=OP.add,
                )
                y0 = seed.bitcast(f32)
                t1 = singles.tile([128, NJ], f32, name=f"rs_t1{name}")
                nc.vector.tensor_tensor(out=t1, in0=y0, in1=y0, op=OP.mult)
                nc.vector.tensor_tensor(out=t1, in0=t1, in1=src, op=OP.mult)
                nc.vector.tensor_scalar(
                    out=t1, in0=t1, scalar1=-0.5, scalar2=1.5, op0=OP.mult, op1=OP.add
                )
                y1 = singles.tile([128, NJ], f32, name=f"rs_y1{name}")
                nc.vector.tensor_tensor(out=y1, in0=y0, in1=t1, op=OP.mult)
                t2 = singles.tile([128, NJ], f32, name=f"rs_t2{name}")
                nc.vector.tensor_tensor(out=t2, in0=y1, in1=y1, op=OP.mult)
                nc.vector.tensor_tensor(out=t2, in0=t2, in1=src, op=OP.mult)
                nc.vector.tensor_scalar(
                    out=t2,
                    in0=t2,
                    scalar1=-0.5 * fold,
                    scalar2=1.5 * fold,
                    op0=OP.mult,
                    op1=OP.add,
                )
                out = singles.tile([128, NJ], f32, name=f"rs_o{name}")
                nc.vector.tensor_tensor(out=out, in0=y1, in1=t2, op=OP.mult)
                return out

            srinv = rsqrt2(ss, fold=SA, name="n")  # SA/||x_row||

            # x_n bf16 (Pool) -> PE transposes -> fp8 x^T (DVE cast)
            xnb = singles.tile([128, NJ, D], bf16)
            xT = singles.tile([128, 2, NLOC], f8)
            for j in range(NJ):
                nc.gpsimd.tensor_tensor(
                    out=xnb[:, j, :],
                    in0=xt[:, j, :],
                    in1=srinv[:, j : j + 1].to_broadcast((128, D)),
                    op=OP.mult,
                )
                for dc in range(2):
                    nc.tensor.transpose(
                        out=ptx[:, dc * NJ + j, :],
                        in_=xnb[:, j, dc * 128 : (dc + 1) * 128],
                        identity=ident,
                    )
                nc.vector.tensor_copy(
                    out=xT[:, :, j * 128 : (j + 1) * 128],
                    in_=ptx[:, j :: NJ, :],
                )

            # ---------------- target-score path ----------------
            traw = singles.tile([128, NJ], f32)
            tprod = singles.tile([128, D], bf16)
            for j in range(NJ):
                nc.vector.scalar_tensor_tensor(
                    out=tprod,
                    in0=xnb[:, j, :],
                    scalar=1.0,
                    in1=wg[:, j, :],
                    op0=OP.mult,
                    op1=OP.mult,
                    accum_out=traw[:, j : j + 1],
                )
            sclip = S * (1.0 - EPS)
            tcl = singles.tile([128, NJ], f32)
            nc.vector.tensor_scalar(
                out=tcl, in0=traw, scalar1=-sclip, scalar2=sclip, op0=OP.max, op1=OP.min
            )
            usq = singles.tile([128, NJ], f32)  # S^2 - t_s^2
            nc.vector.tensor_tensor(out=usq, in0=tcl, in1=tcl, op=OP.mult)
            nc.vector.tensor_scalar(
                out=usq, in0=usq, scalar1=-1.0, scalar2=S * S, op0=OP.mult, op1=OP.add
            )
            # rtm = -sinM * sqrt(usq) = usq * (-sinM * rsqrt(usq))
            rsu = rsqrt2(usq, fold=-math.sin(MARGIN), name="u")
            rtm = singles.tile([128, NJ], f32)
            nc.vector.tensor_tensor(out=rtm, in0=usq, in1=rsu, op=OP.mult)
            numer = singles.tile([128, NJ], f32)
            nc.vector.scalar_tensor_tensor(
                out=numer,
                in0=tcl,
                scalar=math.cos(MARGIN),
                in1=rtm,
                op0=OP.mult,
                op1=OP.add,
            )

            # ---------------- main loop: j outer, chunk pairs inner ----------
            acc = singles.tile([128, NJ, NPAIR], f32)
            edump = singles.tile([128, 2 * CW], bf16)
            exp_num = singles.tile([128, NJ], f32)
            exp_st = singles.tile([128, NJ], f32)

            def fill_chunk(j, c):
                cb = c * CW
                if WIDE_MM:
                    nsub, sw = 2, 512
                else:
                    nsub, sw = 4, 256
                for s_ in range(nsub):
                    nc.tensor.matmul(
                        out=pm[:, c % 4, s_ * sw : (s_ + 1) * sw],
                        lhsT=xT[:, :, j * 128 : (j + 1) * 128],
                        rhs=wt[:, :, cb + s_ * sw : cb + (s_ + 1) * sw],
                        start=True,
                        stop=True,
                        perf_mode=DR,
                        skip_group_check=True,
                    )

            def drain_pair(j, p):
                p0 = (2 * p) % 4
                src = pm[:, p0 : p0 + 2, :]
                if PAIR_PLAN[j][p] == "A":
                    nc.scalar.activation(
                        out=edump,
                        in_=src,
                        func=AF.Exp,
                        accum_out=acc[:, j, p : p + 1],
                    )
                else:
                    idump = idpool.tile([128, 2 * CW], i32, tag="id")
                    nc.vector.tensor_scalar(
                        out=idump,
                        in0=src,
                        scalar1=AEXP,
                        scalar2=BEXP,
                        op0=OP.mult,
                        op1=OP.add,
                    )
                    nc.vector.tensor_reduce(
                        out=acc[:, j, p : p + 1],
                        in_=idump[:, :].bitcast(f32),
                        axis=mybir.AxisListType.X,
                        op=OP.add,
                    )

            for j in range(NJ):
                for c in range(NCH):
                    fill_chunk(j, c)
                    if c % 2 == 1:
                        drain_pair(j, c // 2)
                    if j == 0 and c == 3:
                        # slot into the ACT stream once traw is ready
                        nc.scalar.activation(out=exp_num, in_=numer, func=AF.Exp)
                        nc.scalar.activation(out=exp_st, in_=tcl, func=AF.Exp)

            # ---------------- combine ----------------
            dnum = singles.tile([128, NJ], f32)  # exp(numer) - exp(t_s)
            nc.vector.tensor_tensor(out=dnum, in0=exp_num, in1=exp_st, op=OP.subtract)
            rowsum = singles.tile([128, NJ], f32)
            nc.vector.tensor_reduce(
                out=rowsum, in_=acc, axis=mybir.AxisListType.X, op=OP.add
            )
            denom = singles.tile([128, NJ], f32)
            nc.vector.scalar_tensor_tensor(
                out=denom,
                in0=rowsum,
                scalar=-float(NPAD),
                in1=dnum,
                op0=OP.add,
                op1=OP.add,
            )
            logd = singles.tile([128, NJ], f32)
            nc.scalar.activation(out=logd, in_=denom, func=AF.Ln)
            Lt = singles.tile([128, NJ], f32)
            nc.vector.tensor_tensor(out=Lt, in0=numer, in1=logd, op=OP.subtract)
            Lrow = singles.tile([128, 1], f32)
            nc.vector.tensor_reduce(
                out=Lrow, in_=Lt, axis=mybir.AxisListType.X, op=OP.add
            )
            ones = singles.tile([128, 1], f32)
            nc.vector.memset(ones, 1.0)
            nc.tensor.matmul(
                out=pm[0:1, 3, 0:1], lhsT=Lrow, rhs=ones, start=True, stop=True
            )
            Lp = singles.tile([1, 1], f32)
            nc.vector.tensor_copy(out=Lp, in_=pm[0:1, 3, 0:1])
            nc.sync.dma_start(out=out_ext[:, :], in_=Lp)

    nc.finalize()
    return nc


def _get_nc():
    if "nc" not in _CACHE:
        _CACHE["nc"] = _build()
    return _CACHE["nc"]


def prepare_in_maps(x, W, target):
    import ml_dtypes

    f8 = ml_dtypes.float8_e4m3fn
    bf = ml_dtypes.bfloat16

    x = np.asarray(x, dtype=np.float32)
    W = np.asarray(W, dtype=np.float32)
    tgt = np.asarray(target).astype(np.int64).reshape(N)

    ws = W * np.float32(SB)
    # W^T in [partition(=d%128), plane(=d//128), class] fp8 layout, zero-padded
    wt = np.zeros((128, 2, CP), dtype=f8)
    wt[:, :, :C] = ws.T.reshape(2, 128, C).transpose(1, 0, 2).astype(f8)
    wgather = ws[tgt].astype(bf)  # [N, D] bf16

    in_maps = []
    for c in range(NCORES):
        xs = x[c * NLOC : (c + 1) * NLOC]
        wgs = wgather[c * NLOC : (c + 1) * NLOC]
        in_maps.append(
            {
                "x": np.ascontiguousarray(xs.reshape(NJ, 128, D).transpose(1, 0, 2)),
                "wt": wt,
                "wg": np.ascontiguousarray(wgs.reshape(NJ, 128, D).transpose(1, 0, 2)),
            }
        )
    return in_maps


def kernel(x, W, target):
    from concourse.bass_utils import run_bass_kernel_spmd

    nc = _get_nc()
    in_maps = prepare_in_maps(x, W, target)
    res = run_bass_kernel_spmd(nc, in_maps, core_ids=list(range(NCORES)))
    parts = np.stack(
        [res.results[i]["out"].astype(np.float32).reshape(()) for i in range(NCORES)]
    )
    total = np.sum(parts, dtype=np.float32)
    return np.float32(-(total / np.float32(N)))


# revision 12
# speedup vs baseline: 1.1701x; 1.1701x over previous
"""ArcFace (AngularPenaltySMLoss) fused loss kernel for 8 Trainium2 NeuronCores.

Strategy: data-parallel over rows N (each core owns N/8 = 1024 rows of x,
streams the full W). Key design points:

  1. fp8(e4m3) DoubleRow matmul: host uploads W^T [128, 2, 10240] fp8 (scaled
     by SB, zero-padded 10000->10240) and x_n^T [128, 2, 1024] fp8 (row-
     normalized, scaled by SA; SA*SB = S = 30).  Each matmul contracts the
     full K=256 (two 128-planes) in one instruction streaming 2048 rhs rows,
     one PSUM slot per instruction.  The 240 zero-pad classes contribute
     exactly exp(0)=1 each on ACT-drained pairs (pad pairs are always 'A');
     the tail subtracts the constant 240.
  2. j-outer main loop over a manually managed 4-slot PSUM ring
     (pm[128, 4, 1024] f32 = all 16KB), slot = global_chunk % 4, so
     consecutive drain PAIRS alternate between slot pairs (0,1) and (2,3) and
     the two drain engines run concurrently with PE refills.  Per row-block j,
     10 class-chunks of 1024 drain in pairs via one wide [128, 2048]
     instruction: 'A' pairs = ACT exp with fused row-sum accum_out; 'V' pairs
     = DVE Schraudolph bit-trick exp (int32(A*v+B) = f32 bit pattern of
     ~exp(v)) + DVE reduce of the bitcast (B tuned for ~zero exp-weighted
     mean error; loss error ~1e-4 vs the 2e-2 gate).
  3. ACT runs ONLY Exp until the single final Ln (one act-table switch at the
     tail).  sqrt(S^2-t^2) in the numerator uses the Quake rsqrt bit trick +
     2 Newton steps on DVE.
  4. Target path: host pre-gathers (W*SB)[target] as bf16 (data movement
     only); the on-device dot x_n.Wg runs on DVE with fused accumulation;
     numerator = cosM*t_s - sinM*sqrt(S^2-t_s^2) on scaled t_s = S*t.
  5. Per-core partial sum of L_i; host combines 8 scalars.
"""

import math

import numpy as np

S = 30.0
MARGIN = 0.3
EPS = 1e-7
N, D, C = 8192, 256, 10000
NCORES = 8
NLOC = N // NCORES  # 1024 rows per core
NJ = NLOC // 128  # 8 row-chunks of 128 partitions
CP = 10240  # padded class count
CW = 1024  # class-chunk width (one PSUM slot)
NCH = CP // CW  # 10 chunks per row-block
NPAIR = NCH // 2  # 5 drain pairs per row-block
NPAD = CP - C  # 240 zero-pad classes -> exp contributes exactly NPAD
SA = 8.0  # fp8 scale folded into normalized x
SB = 3.75  # fp8 scale folded into W  (SA*SB = S)

# Schraudolph exp constants (f32 domain, int32 bit pattern), B tuned for
# zero exp-weighted mean error: B = 127*2^23 - round(0.0562*2^23)
AEXP = 12102203.0
BEXP = 1064881816.0
RSQRT_MAGIC = 1597463007.0  # 0x5f3759df

# Drain-engine plan per (j, pair): 'A' = ACT, 'V' = DVE Schraudolph.
# Last pair of each j must be 'A' (owns the zero-pad classes). 28A:12V
# balances measured ACT (~2.1us/pair) vs DVE (~4.4us/pair + misc).
PAIR_PLAN = [
    "AVAAA",
    "AVAVA",
    "AVAAA",
    "AVAVA",
    "AVAAA",
    "AVAVA",
    "AVAVA",
    "AVAAA",
]

_CACHE = {}


def _build():
    import concourse.bass as bass  # noqa: F401
    import concourse.mybir as mybir
    import concourse.tile as tile
    from concourse import bacc

    f32 = mybir.dt.float32
    bf16 = mybir.dt.bfloat16
    f8 = mybir.dt.float8e4
    i32 = mybir.dt.int32
    AF = mybir.ActivationFunctionType
    OP = mybir.AluOpType
    DR = mybir.MatmulPerfMode.DoubleRow

    nc = bacc.Bacc()
    xT_ext = nc.declare_dram_parameter("xT", [128, 2, NLOC], f8, isOutput=False)
    wt_ext = nc.declare_dram_parameter("wt", [128, 2, CP], f8, isOutput=False)
    xnb_ext = nc.declare_dram_parameter("xnb", [128, NJ, D], bf16, isOutput=False)
    wg_ext = nc.declare_dram_parameter("wg", [128, NJ, D], bf16, isOutput=False)
    out_ext = nc.declare_dram_parameter("out", [1, 1], f32, isOutput=True)

    with tile.TileContext(nc) as tc:
        with (
            tc.tile_pool(name="singles", bufs=1) as singles,
            tc.tile_pool(name="idpool", bufs=2) as idpool,
            tc.tile_pool(name="pmain", bufs=1, space="PSUM") as psum_main,
        ):
            # the whole PSUM as a manually phased 4-slot ring
            pm = psum_main.tile([128, 4, CW], f32)

            # ---------------- loads (j=0 critical path first) ------------
            xT = singles.tile([128, 2, NLOC], f8)
            wt = singles.tile([128, 2, CP], f8)
            xnb = singles.tile([128, NJ, D], bf16)
            wg = singles.tile([128, NJ, D], bf16)
            nc.sync.dma_start(out=xT, in_=xT_ext[:, :, :])
            nc.sync.dma_start(out=wt[:, :, 0:2048], in_=wt_ext[:, :, 0:2048])
            nc.sync.dma_start(out=xnb, in_=xnb_ext[:, :, :])
            nc.sync.dma_start(out=wg, in_=wg_ext[:, :, :])
            for r in range(1, NPAIR):
                c0 = r * 2048
                nc.sync.dma_start(
                    out=wt[:, :, c0 : c0 + 2048], in_=wt_ext[:, :, c0 : c0 + 2048]
                )

            # ---------------- target-score path (DVE, off critical path) --
            traw = singles.tile([128, NJ], f32)
            tprod = singles.tile([128, D], bf16)

            def tdot(j):
                nc.vector.scalar_tensor_tensor(
                    out=tprod,
                    in0=xnb[:, j, :],
                    scalar=1.0,
                    in1=wg[:, j, :],
                    op0=OP.mult,
                    op1=OP.mult,
                    accum_out=traw[:, j : j + 1],
                )

            rs_seed = singles.tile([128, NJ], i32)
            rs_t1 = singles.tile([128, NJ], f32)
            rs_y1 = singles.tile([128, NJ], f32)
            rs_t2 = singles.tile([128, NJ], f32)

            def rsqrt2(src, dst, fold=1.0):
                # Quake rsqrt + 2 Newton iterations; dst = fold/sqrt(src)
                nc.vector.tensor_scalar(
                    out=rs_seed,
                    in0=src.bitcast(i32),
                    scalar1=-0.5,
                    scalar2=RSQRT_MAGIC,
                    op0=OP.mult,
                    op1=OP.add,
                )
                y0 = rs_seed.bitcast(f32)
                nc.vector.tensor_tensor(out=rs_t1, in0=y0, in1=y0, op=OP.mult)
                nc.vector.tensor_tensor(out=rs_t1, in0=rs_t1, in1=src, op=OP.mult)
                nc.vector.tensor_scalar(
                    out=rs_t1, in0=rs_t1, scalar1=-0.5, scalar2=1.5,
                    op0=OP.mult, op1=OP.add,
                )
                nc.vector.tensor_tensor(out=rs_y1, in0=y0, in1=rs_t1, op=OP.mult)
                nc.vector.tensor_tensor(out=rs_t2, in0=rs_y1, in1=rs_y1, op=OP.mult)
                nc.vector.tensor_tensor(out=rs_t2, in0=rs_t2, in1=src, op=OP.mult)
                nc.vector.tensor_scalar(
                    out=rs_t2, in0=rs_t2, scalar1=-0.5 * fold, scalar2=1.5 * fold,
                    op0=OP.mult, op1=OP.add,
                )
                nc.vector.tensor_tensor(out=dst, in0=rs_y1, in1=rs_t2, op=OP.mult)

            def numer_chain():
                sclip = S * (1.0 - EPS)
                nc.vector.tensor_scalar(
                    out=tcl, in0=traw, scalar1=-sclip, scalar2=sclip,
                    op0=OP.max, op1=OP.min,
                )
                nc.vector.tensor_tensor(out=usq, in0=tcl, in1=tcl, op=OP.mult)
                nc.vector.tensor_scalar(
                    out=usq, in0=usq, scalar1=-1.0, scalar2=S * S,
                    op0=OP.mult, op1=OP.add,
                )
                # rtm = -sinM*sqrt(usq) = usq * (-sinM * rsqrt(usq))
                rsqrt2(usq, rsu, fold=-math.sin(MARGIN))
                nc.vector.tensor_tensor(out=rtm, in0=usq, in1=rsu, op=OP.mult)
                nc.vector.scalar_tensor_tensor(
                    out=numer, in0=tcl, scalar=math.cos(MARGIN), in1=rtm,
                    op0=OP.mult, op1=OP.add,
                )

            tcl = singles.tile([128, NJ], f32)
            usq = singles.tile([128, NJ], f32)
            rsu = singles.tile([128, NJ], f32)
            rtm = singles.tile([128, NJ], f32)
            numer = singles.tile([128, NJ], f32)
            exp_num = singles.tile([128, NJ], f32)
            exp_st = singles.tile([128, NJ], f32)

            # ---------------- main loop: j outer, chunk pairs inner --------
            acc = singles.tile([128, NJ, NPAIR], f32)
            edump = singles.tile([128, 2 * CW], bf16)

            def drain_pair(j, p, slot0):
                src = pm[:, slot0 : slot0 + 2, :]
                if PAIR_PLAN[j][p] == "A":
                    nc.scalar.activation(
                        out=edump,
                        in_=src,
                        func=AF.Exp,
                        accum_out=acc[:, j, p : p + 1],
                    )
                else:
                    idump = idpool.tile([128, 2 * CW], i32, tag="id")
                    nc.vector.tensor_scalar(
                        out=idump,
                        in0=src,
                        scalar1=AEXP,
                        scalar2=BEXP,
                        op0=OP.mult,
                        op1=OP.add,
                    )
                    nc.vector.tensor_reduce(
                        out=acc[:, j, p : p + 1],
                        in_=idump[:, :].bitcast(f32),
                        axis=mybir.AxisListType.X,
                        op=OP.add,
                    )

            g = 0  # global chunk counter -> PSUM slot phase
            for j in range(NJ):
                for c in range(NCH):
                    for s_ in range(2):
                        nc.tensor.matmul(
                            out=pm[:, g % 4, s_ * 512 : (s_ + 1) * 512],
                            lhsT=xT[:, :, j * 128 : (j + 1) * 128],
                            rhs=wt[:, :, c * CW + s_ * 512 : c * CW + (s_ + 1) * 512],
                            start=True,
                            stop=True,
                            perf_mode=DR,
                            skip_group_check=True,
                        )
                    g += 1
                    if c % 2 == 1:
                        drain_pair(j, c // 2, (g - 2) % 4)
                if j == 0:
                    # DVE target-path work slots in behind the first sweep
                    for jj in range(NJ):
                        tdot(jj)
                    numer_chain()
                    nc.scalar.activation(out=exp_num, in_=numer, func=AF.Exp)
                    nc.scalar.activation(out=exp_st, in_=tcl, func=AF.Exp)

            # ---------------- combine ----------------
            dnum = singles.tile([128, NJ], f32)  # exp(numer) - exp(t_s)
            nc.vector.tensor_tensor(out=dnum, in0=exp_num, in1=exp_st, op=OP.subtract)
            rowsum = singles.tile([128, NJ], f32)
            nc.vector.tensor_reduce(
                out=rowsum, in_=acc, axis=mybir.AxisListType.X, op=OP.add
            )
            denom = singles.tile([128, NJ], f32)
            nc.vector.scalar_tensor_tensor(
                out=denom,
                in0=rowsum,
                scalar=-float(NPAD),
                in1=dnum,
                op0=OP.add,
                op1=OP.add,
            )
            logd = singles.tile([128, NJ], f32)
            nc.scalar.activation(out=logd, in_=denom, func=AF.Ln)
            Lt = singles.tile([128, NJ], f32)
            nc.vector.tensor_tensor(out=Lt, in0=numer, in1=logd, op=OP.subtract)
            Lrow = singles.tile([128, 1], f32)
            nc.vector.tensor_reduce(
                out=Lrow, in_=Lt, axis=mybir.AxisListType.X, op=OP.add
            )
            ones = singles.tile([128, 1], f32)
            nc.vector.memset(ones, 1.0)
            nc.tensor.matmul(
                out=pm[0:1, 3, 0:1], lhsT=Lrow, rhs=ones, start=True, stop=True
            )
            Lp = singles.tile([1, 1], f32)
            nc.vector.tensor_copy(out=Lp, in_=pm[0:1, 3, 0:1])
            nc.sync.dma_start(out=out_ext[:, :], in_=Lp)

    nc.finalize()
    return nc


def _get_nc():
    if "nc" not in _CACHE:
        _CACHE["nc"] = _build()
    return _CACHE["nc"]


def prepare_in_maps(x, W, target):
    import ml_dtypes

    f8 = ml_dtypes.float8_e4m3fn
    bf = ml_dtypes.bfloat16

    x = np.asarray(x, dtype=np.float32)
    W = np.asarray(W, dtype=np.float32)
    tgt = np.asarray(target).astype(np.int64).reshape(N)

    xn = x / np.linalg.norm(x, axis=1, keepdims=True)
    xna = (xn * np.float32(SA)).astype(np.float32)

    ws = W * np.float32(SB)
    # W^T in [partition(=d%128), plane(=d//128), class] fp8 layout, zero-padded
    wt = np.zeros((128, 2, CP), dtype=f8)
    wt[:, :, :C] = ws.T.reshape(2, 128, C).transpose(1, 0, 2).astype(f8)
    wgather = ws[tgt].astype(bf)  # [N, D] bf16

    in_maps = []
    for c in range(NCORES):
        sl = slice(c * NLOC, (c + 1) * NLOC)
        xs, wgs = xna[sl], wgather[sl]
        in_maps.append(
            {
                # x_n^T fp8 [d%128, d//128, row]
                "xT": np.ascontiguousarray(
                    xs.T.reshape(2, 128, NLOC).transpose(1, 0, 2).astype(f8)
                ),
                "wt": wt,
                # x_n bf16 [row%128, row//128, d] (for the target dot)
                "xnb": np.ascontiguousarray(
                    xs.reshape(NJ, 128, D).transpose(1, 0, 2).astype(bf)
                ),
                "wg": np.ascontiguousarray(wgs.reshape(NJ, 128, D).transpose(1, 0, 2)),
            }
        )
    return in_maps


def kernel(x, W, target):
    from concourse.bass_utils import run_bass_kernel_spmd

    nc = _get_nc()
    in_maps = prepare_in_maps(x, W, target)
    res = run_bass_kernel_spmd(nc, in_maps, core_ids=list(range(NCORES)))
    parts = np.stack(
        [res.results[i]["out"].astype(np.float32).reshape(()) for i in range(NCORES)]
    )
    total = np.sum(parts, dtype=np.float32)
    return np.float32(-(total / np.float32(N)))


# revision 16
# speedup vs baseline: 1.2803x; 1.0942x over previous
"""ArcFace (AngularPenaltySMLoss) fused loss kernel for 8 Trainium2 NeuronCores.

Strategy: data-parallel over rows N (each core owns N/8 = 1024 rows of x,
streams the full W). Key design points:

  1. fp8(e4m3) DoubleRow matmul: host uploads W^T [128, 2, 10240] fp8 (scaled
     by SB, zero-padded 10000->10240) and x_n^T [128, 2, 1024] fp8 (row-
     normalized, scaled by SA; SA*SB = S = 30).  Each matmul contracts the
     full K=256 (two 128-planes) in one instruction streaming 2048 rhs rows,
     one PSUM slot per instruction.  The 240 zero-pad classes contribute
     exactly exp(0)=1 each on ACT-drained pairs (pad pairs are always 'A');
     the tail subtracts the constant 240.
  2. j-outer main loop over a manually managed 4-slot PSUM ring
     (pm[128, 4, 1024] f32 = all 16KB), slot = global_chunk % 4, so
     consecutive drain PAIRS alternate between slot pairs (0,1) and (2,3) and
     the two drain engines run concurrently with PE refills.  Per row-block j,
     10 class-chunks of 1024 drain in pairs via one wide [128, 2048]
     instruction: 'A' pairs = ACT exp with fused row-sum accum_out; 'V' pairs
     = DVE Schraudolph bit-trick exp (int32(A*v+B) = f32 bit pattern of
     ~exp(v)) + DVE reduce of the bitcast (B tuned for ~zero exp-weighted
     mean error; loss error ~1e-4 vs the 2e-2 gate).
  3. ACT runs ONLY Exp until the single final Ln (one act-table switch at the
     tail).  sqrt(S^2-t^2) in the numerator uses the Quake rsqrt bit trick +
     2 Newton steps on DVE.
  4. Target path: host pre-gathers (W*SB)[target] as bf16 (data movement
     only); the on-device dot x_n.Wg runs on DVE with fused accumulation;
     numerator = cosM*t_s - sinM*sqrt(S^2-t_s^2) on scaled t_s = S*t.
  5. Per-core partial sum of L_i; host combines 8 scalars.
"""

import math

import numpy as np

S = 30.0
MARGIN = 0.3
EPS = 1e-7
N, D, C = 8192, 256, 10000
NCORES = 8
NLOC = N // NCORES  # 1024 rows per core
NJ = NLOC // 128  # 8 row-chunks of 128 partitions
CP = 10240  # padded class count
CW = 1024  # class-chunk width (one PSUM slot)
NCH = CP // CW  # 10 chunks per row-block
NPAIR = NCH // 2  # 5 drain pairs per row-block
NPAD = CP - C  # 240 zero-pad classes -> exp contributes exactly NPAD
SA = 8.0  # fp8 scale folded into normalized x
SB = 3.75  # fp8 scale folded into W  (SA*SB = S)

# Schraudolph exp constants (f32 domain, int32 bit pattern), B tuned for
# zero exp-weighted mean error: B = 127*2^23 - round(0.0562*2^23)
AEXP = 12102203.0
BEXP = 1064881816.0
RSQRT_MAGIC = 1597463007.0  # 0x5f3759df

# Drain-engine plan per (j, pair): 'A' = ACT, 'V' = DVE Schraudolph,
# 'H' = hybrid (first 1024-chunk on ACT, second on DVE).  A uniform mix per
# row-block keeps both drain engines busy every sweep instead of
# alternating the bottleneck.  Last pair of each j must be 'A' or 'H'
# (ACT half owns no pad; pads are in pair 4 = 'A').
PAIR_PLAN = [
    "AVAHA",
    "AVAHA",
    "AVAHA",
    "AVAHA",
    "AVAHA",
    "AVAHA",
    "AVAHA",
    "AVAHA",
]

_CACHE = {}


def _build():
    import concourse.bass as bass  # noqa: F401
    import concourse.mybir as mybir
    import concourse.tile as tile
    from concourse import bacc

    f32 = mybir.dt.float32
    bf16 = mybir.dt.bfloat16
    f8 = mybir.dt.float8e4
    i32 = mybir.dt.int32
    AF = mybir.ActivationFunctionType
    OP = mybir.AluOpType
    DR = mybir.MatmulPerfMode.DoubleRow

    nc = bacc.Bacc()
    xT_ext = nc.declare_dram_parameter("xT", [128, 2, NLOC], f8, isOutput=False)
    wt_ext = nc.declare_dram_parameter("wt", [128, 2, CP], f8, isOutput=False)
    xnb_ext = nc.declare_dram_parameter("xnb", [128, NJ, D], bf16, isOutput=False)
    wg_ext = nc.declare_dram_parameter("wg", [128, NJ, D], bf16, isOutput=False)
    out_ext = nc.declare_dram_parameter("out", [1, 1], f32, isOutput=True)

    with tile.TileContext(nc) as tc:
        with (
            tc.tile_pool(name="singles", bufs=1) as singles,
            tc.tile_pool(name="idpool", bufs=2) as idpool,
            tc.tile_pool(name="pmain", bufs=1, space="PSUM") as psum_main,
        ):
            # the whole PSUM as a manually phased 4-slot ring
            pm = psum_main.tile([128, 4, CW], f32)

            # ---------------- loads (j=0 critical path first) ------------
            xT = singles.tile([128, 2, NLOC], f8)
            wt = singles.tile([128, 2, CP], f8)
            xnb = singles.tile([128, NJ, D], bf16)
            wg = singles.tile([128, NJ, D], bf16)
            # two parallel HWDGE queues: SP carries W rounds 0,2,3,4 + the
            # target-path tensors; the Activation queue carries x^T + W round 1
            nc.scalar.dma_start(out=xT, in_=xT_ext[:, :, :])
            nc.sync.dma_start(out=wt[:, :, 0:2048], in_=wt_ext[:, :, 0:2048])
            nc.scalar.dma_start(out=wt[:, :, 2048:4096], in_=wt_ext[:, :, 2048:4096])
            for r in range(2, NPAIR):
                c0 = r * 2048
                nc.sync.dma_start(
                    out=wt[:, :, c0 : c0 + 2048], in_=wt_ext[:, :, c0 : c0 + 2048]
                )
            nc.sync.dma_start(out=xnb, in_=xnb_ext[:, :, :])
            nc.sync.dma_start(out=wg, in_=wg_ext[:, :, :])

            # ---------------- target-score path (DVE, off critical path) --
            traw = singles.tile([128, NJ], f32)
            tprod = singles.tile([128, D], bf16)

            def tdot(j):
                nc.vector.scalar_tensor_tensor(
                    out=tprod,
                    in0=xnb[:, j, :],
                    scalar=1.0,
                    in1=wg[:, j, :],
                    op0=OP.mult,
                    op1=OP.mult,
                    accum_out=traw[:, j : j + 1],
                )

            rs_seed = singles.tile([128, NJ], i32)
            rs_t1 = singles.tile([128, NJ], f32)
            rs_y1 = singles.tile([128, NJ], f32)
            rs_t2 = singles.tile([128, NJ], f32)

            def rsqrt2(src, dst, fold=1.0):
                # Quake rsqrt + 2 Newton iterations; dst = fold/sqrt(src)
                nc.vector.tensor_scalar(
                    out=rs_seed,
                    in0=src.bitcast(i32),
                    scalar1=-0.5,
                    scalar2=RSQRT_MAGIC,
                    op0=OP.mult,
                    op1=OP.add,
                )
                y0 = rs_seed.bitcast(f32)
                nc.vector.tensor_tensor(out=rs_t1, in0=y0, in1=y0, op=OP.mult)
                nc.vector.tensor_tensor(out=rs_t1, in0=rs_t1, in1=src, op=OP.mult)
                nc.vector.tensor_scalar(
                    out=rs_t1, in0=rs_t1, scalar1=-0.5, scalar2=1.5,
                    op0=OP.mult, op1=OP.add,
                )
                nc.vector.tensor_tensor(out=rs_y1, in0=y0, in1=rs_t1, op=OP.mult)
                nc.vector.tensor_tensor(out=rs_t2, in0=rs_y1, in1=rs_y1, op=OP.mult)
                nc.vector.tensor_tensor(out=rs_t2, in0=rs_t2, in1=src, op=OP.mult)
                nc.vector.tensor_scalar(
                    out=rs_t2, in0=rs_t2, scalar1=-0.5 * fold, scalar2=1.5 * fold,
                    op0=OP.mult, op1=OP.add,
                )
                nc.vector.tensor_tensor(out=dst, in0=rs_y1, in1=rs_t2, op=OP.mult)

            def numer_chain():
                sclip = S * (1.0 - EPS)
                nc.vector.tensor_scalar(
                    out=tcl, in0=traw, scalar1=-sclip, scalar2=sclip,
                    op0=OP.max, op1=OP.min,
                )
                nc.vector.tensor_tensor(out=usq, in0=tcl, in1=tcl, op=OP.mult)
                nc.vector.tensor_scalar(
                    out=usq, in0=usq, scalar1=-1.0, scalar2=S * S,
                    op0=OP.mult, op1=OP.add,
                )
                # rtm = -sinM*sqrt(usq) = usq * (-sinM * rsqrt(usq))
                rsqrt2(usq, rsu, fold=-math.sin(MARGIN))
                nc.vector.tensor_tensor(out=rtm, in0=usq, in1=rsu, op=OP.mult)
                nc.vector.scalar_tensor_tensor(
                    out=numer, in0=tcl, scalar=math.cos(MARGIN), in1=rtm,
                    op0=OP.mult, op1=OP.add,
                )

            tcl = singles.tile([128, NJ], f32)
            usq = singles.tile([128, NJ], f32)
            rsu = singles.tile([128, NJ], f32)
            rtm = singles.tile([128, NJ], f32)
            numer = singles.tile([128, NJ], f32)
            exp_num = singles.tile([128, NJ], f32)
            exp_st = singles.tile([128, NJ], f32)

            # ---------------- main loop: j outer, chunk pairs inner --------
            # acc column NPAIR holds the hybrid pair's DVE half
            acc = singles.tile([128, NJ, NPAIR + 1], f32)
            edump = singles.tile([128, 2 * CW], bf16)

            def sch_chunk(src, accslot):
                idump = idpool.tile([128, 2 * CW], i32, tag="id")
                nelem = src.free_size()
                nc.vector.tensor_scalar(
                    out=idump[:, :nelem],
                    in0=src,
                    scalar1=AEXP,
                    scalar2=BEXP,
                    op0=OP.mult,
                    op1=OP.add,
                )
                nc.vector.tensor_reduce(
                    out=accslot,
                    in_=idump[:, :nelem].bitcast(f32),
                    axis=mybir.AxisListType.X,
                    op=OP.add,
                )

            def drain_pair(j, p, slot0):
                kind = PAIR_PLAN[j][p]
                if kind == "A":
                    nc.scalar.activation(
                        out=edump,
                        in_=pm[:, slot0 : slot0 + 2, :],
                        func=AF.Exp,
                        accum_out=acc[:, j, p : p + 1],
                    )
                elif kind == "V":
                    sch_chunk(pm[:, slot0 : slot0 + 2, :], acc[:, j, p : p + 1])
                else:  # hybrid: ACT takes the first slot, DVE the second
                    nc.scalar.activation(
                        out=edump[:, :CW],
                        in_=pm[:, slot0, :],
                        func=AF.Exp,
                        accum_out=acc[:, j, p : p + 1],
                    )
                    sch_chunk(pm[:, slot0 + 1, :], acc[:, j, NPAIR : NPAIR + 1])

            g = 0  # global chunk counter -> PSUM slot phase
            for j in range(NJ):
                for c in range(NCH):
                    for s_ in range(2):
                        nc.tensor.matmul(
                            out=pm[:, g % 4, s_ * 512 : (s_ + 1) * 512],
                            lhsT=xT[:, :, j * 128 : (j + 1) * 128],
                            rhs=wt[:, :, c * CW + s_ * 512 : c * CW + (s_ + 1) * 512],
                            start=True,
                            stop=True,
                            perf_mode=DR,
                            skip_group_check=True,
                        )
                    g += 1
                    if c % 2 == 1:
                        drain_pair(j, c // 2, (g - 2) % 4)
                if j == 0:
                    # DVE target-path work slots in behind the first sweep
                    for jj in range(NJ):
                        tdot(jj)
                    numer_chain()
                elif j == 1:
                    nc.scalar.activation(out=exp_num, in_=numer, func=AF.Exp)
                    nc.scalar.activation(out=exp_st, in_=tcl, func=AF.Exp)

            # ---------------- combine ----------------
            dnum = singles.tile([128, NJ], f32)  # exp(numer) - exp(t_s)
            nc.vector.tensor_tensor(out=dnum, in0=exp_num, in1=exp_st, op=OP.subtract)
            rowsum = singles.tile([128, NJ], f32)
            nc.vector.tensor_reduce(
                out=rowsum, in_=acc, axis=mybir.AxisListType.X, op=OP.add
            )
            denom = singles.tile([128, NJ], f32)
            nc.vector.scalar_tensor_tensor(
                out=denom,
                in0=rowsum,
                scalar=-float(NPAD),
                in1=dnum,
                op0=OP.add,
                op1=OP.add,
            )
            logd = singles.tile([128, NJ], f32)
            nc.scalar.activation(out=logd, in_=denom, func=AF.Ln)
            Lt = singles.tile([128, NJ], f32)
            nc.vector.tensor_tensor(out=Lt, in0=numer, in1=logd, op=OP.subtract)
            Lrow = singles.tile([128, 1], f32)
            nc.vector.tensor_reduce(
                out=Lrow, in_=Lt, axis=mybir.AxisListType.X, op=OP.add
            )
            ones = singles.tile([128, 1], f32)
            nc.vector.memset(ones, 1.0)
            nc.tensor.matmul(
                out=pm[0:1, 3, 0:1], lhsT=Lrow, rhs=ones, start=True, stop=True
            )
            Lp = singles.tile([1, 1], f32)
            nc.vector.tensor_copy(out=Lp, in_=pm[0:1, 3, 0:1])
            nc.sync.dma_start(out=out_ext[:, :], in_=Lp)

    nc.finalize()
    return nc


def _get_nc():
    if "nc" not in _CACHE:
        _CACHE["nc"] = _build()
    return _CACHE["nc"]


def prepare_in_maps(x, W, target):
    import ml_dtypes

    f8 = ml_dtypes.float8_e4m3fn
    bf = ml_dtypes.bfloat16

    x = np.asarray(x, dtype=np.float32)
    W = np.asarray(W, dtype=np.float32)
    tgt = np.asarray(target).astype(np.int64).reshape(N)

    xn = x / np.linalg.norm(x, axis=1, keepdims=True)
    xna = (xn * np.float32(SA)).astype(np.float32)

    ws = W * np.float32(SB)
    # W^T in [partition(=d%128), plane(=d//128), class] fp8 layout, zero-padded
    wt = np.zeros((128, 2, CP), dtype=f8)
    wt[:, :, :C] = ws.T.reshape(2, 128, C).transpose(1, 0, 2).astype(f8)
    wgather = ws[tgt].astype(bf)  # [N, D] bf16

    in_maps = []
    for c in range(NCORES):
        sl = slice(c * NLOC, (c + 1) * NLOC)
        xs, wgs = xna[sl], wgather[sl]
        in_maps.append(
            {
                # x_n^T fp8 [d%128, d//128, row]
                "xT": np.ascontiguousarray(
                    xs.T.reshape(2, 128, NLOC).transpose(1, 0, 2).astype(f8)
                ),
                "wt": wt,
                # x_n bf16 [row%128, row//128, d] (for the target dot)
                "xnb": np.ascontiguousarray(
                    xs.reshape(NJ, 128, D).transpose(1, 0, 2).astype(bf)
                ),
                "wg": np.ascontiguousarray(wgs.reshape(NJ, 128, D).transpose(1, 0, 2)),
            }
        )
    return in_maps


def kernel(x, W, target):
    from concourse.bass_utils import run_bass_kernel_spmd

    nc = _get_nc()
    in_maps = prepare_in_maps(x, W, target)
    res = run_bass_kernel_spmd(nc, in_maps, core_ids=list(range(NCORES)))
    parts = np.stack(
        [res.results[i]["out"].astype(np.float32).reshape(()) for i in range(NCORES)]
    )
    total = np.sum(parts, dtype=np.float32)
    return np.float32(-(total / np.float32(N)))
